# revision 4
# baseline (speedup 1.0000x reference)
"""FourierLinear Trainium2 kernel.

Math: reference computes
    dense[a_f, b_f] = s_f   (sparse scatter, 2048 points into 4096x4096)
    delta_w = Re(ifft2(dense)) * 256
    out = x @ delta_w

ifft2 of a sparse spectrum factors exactly:
    delta_w[k, l] = c * sum_f s_f * cos(2*pi*(k*a_f + l*b_f)/4096),  c = 256/4096^2
                  = c * (Cin @ Cout - Sin @ Sout)        (rank 2*2048 factorization)
with Cin[k,f]=cos(2*pi*k*a_f/4096) etc.  Stacking cos/sin blocks:
    G  [4096(f'), 4096(k)]:  rows 0..2047 = cos(2*pi*k*a_f/4096), rows 2048.. = sin(...)
    H  [4096(f'), 4096(l)]:  rows 0..2047 = c*s_f*cos(2*pi*l*b_f/4096), rows 2048.. = -c*s_f*sin(...)
    delta_w = G.T @ H

Sharding: tensor-parallel over delta_w columns. Core j builds
W_j = G.T @ H[:, 512j:512j+512] and computes out[:, cols_j] = x @ W_j.
No collectives; host concatenates the column blocks.

Matmuls run in float32r (FP22 multiply, fp32 accumulate): 4x the fp32 rate
when the moving free dim >= 256.
"""

import numpy as np

import concourse.bass as bass
import concourse.mybir as mybir
import concourse.tile as tile
from concourse import bacc
from concourse.bass_utils import run_bass_kernel_spmd
from concourse.kernels.tile_matmul import matmul_tile_kernel

N_CORES = 8
IN_F = 4096
OUT_F = 4096
N_FREQ = 2048
FOURIER_SCALE = 256.0
ROWS = 4 * 2048  # flattened batch*seq
COLS = OUT_F // N_CORES  # output columns per core

LAST_RESULTS = None  # test harness introspection (exec_time_ns etc.)

_NC_CACHE = None


def _build_nc():
    f32 = mybir.dt.float32
    f32r = mybir.dt.float32r
    nc = bacc.Bacc(None)
    xT = nc.declare_dram_parameter("xT", [IN_F, ROWS], f32, isOutput=False)
    g = nc.declare_dram_parameter("g", [2 * N_FREQ, IN_F], f32, isOutput=False)
    h = nc.declare_dram_parameter("h", [2 * N_FREQ, COLS], f32, isOutput=False)
    out = nc.declare_dram_parameter("out", [ROWS, COLS], f32, isOutput=True)
    w = nc.dram_tensor("w", [IN_F, COLS], f32)

    with tile.TileContext(nc) as tc:
        # stage 1: W = G.T @ H   [4096, COLS]
        matmul_tile_kernel(
            tc,
            g[:].bitcast(f32r),
            h[:].bitcast(f32r),
            w[:],
        )
        # stage 2: out = x @ W = xT.T @ W   [ROWS, COLS]
        matmul_tile_kernel(
            tc,
            xT[:].bitcast(f32r),
            w[:].bitcast(f32r),
            out[:],
        )
    nc.finalize()  # Bacc: runs compile passes (reg alloc, wait splitting)
    return nc


def _host_prep(x, spectrum, indices):
    a = np.asarray(indices[0], dtype=np.int64)
    b = np.asarray(indices[1], dtype=np.int64)
    s = np.asarray(spectrum, dtype=np.float32).copy()

    # reference scatter uses .set semantics: for duplicate (a,b) pairs only the
    # last write survives; zero out earlier duplicates so the additive
    # factorization matches.
    pair = a * OUT_F + b
    last = {}
    for i, p in enumerate(pair):
        last[p] = i
    if len(last) != len(pair):
        keep = np.zeros(len(pair), dtype=bool)
        keep[list(last.values())] = True
        s = np.where(keep, s, 0.0).astype(np.float32)

    k = np.arange(IN_F, dtype=np.int64)
    ang = np.arange(IN_F, dtype=np.float64) * (2.0 * np.pi / IN_F)
    cos_t = np.cos(ang).astype(np.float32)
    sin_t = np.sin(ang).astype(np.float32)

    pa = (a[:, None] * k[None, :]) & (IN_F - 1)
    G = np.empty((2 * N_FREQ, IN_F), np.float32)
    G[:N_FREQ] = cos_t[pa]
    G[N_FREQ:] = sin_t[pa]
    del pa

    c = np.float32(FOURIER_SCALE / (IN_F * OUT_F))
    sc = (s * c)[:, None]
    pb = (b[:, None] * k[None, :]) & (IN_F - 1)
    H = np.empty((2 * N_FREQ, OUT_F), np.float32)
    H[:N_FREQ] = sc * cos_t[pb]
    H[N_FREQ:] = (-sc) * sin_t[pb]
    del pb

    xT = np.ascontiguousarray(np.asarray(x, dtype=np.float32).reshape(ROWS, IN_F).T)
    return xT, G, H


def kernel(x, spectrum, indices):
    global _NC_CACHE, LAST_RESULTS
    xT, G, H = _host_prep(x, spectrum, indices)

    if _NC_CACHE is None:
        _NC_CACHE = _build_nc()
    nc = _NC_CACHE

    in_maps = [
        {"xT": xT, "g": G, "h": np.ascontiguousarray(H[:, j * COLS : (j + 1) * COLS])}
        for j in range(N_CORES)
    ]
    res = run_bass_kernel_spmd(nc, in_maps, list(range(N_CORES)))
    LAST_RESULTS = res
    out = np.concatenate([res.results[j]["out"] for j in range(N_CORES)], axis=1)
    return out.reshape(np.asarray(x).shape[:-1] + (OUT_F,)).astype(np.float32)
